# revision 11
# baseline (speedup 1.0000x reference)
import sys
import numpy as np

for _p in ("/root/.axon_site", "/root/.axon_site/_ro/trn_rl_repo",
           "/root/.axon_site/_ro/pypackages", "/opt/trn_rl_repo"):
    if _p not in sys.path:
        sys.path.append(_p)

import ml_dtypes

B, S, D, H, DFF = 4, 2048, 1024, 16, 4096
DK = D // H
EPS = 1e-9
N_CORES = 8
M = S // 2
NB = ml_dtypes.bfloat16

KI = D // 128
KT = S // 128
QC = M // 512
VST = DK + 1


def build(mask_has_zeros: bool, reps: int = 1, ablate=None, ndev=N_CORES):
    import concourse.bass as bass
    import concourse.mybir as mybir
    import concourse.tile as tile
    from concourse import bacc
    import contextlib

    BF = mybir.dt.bfloat16
    F32 = mybir.dt.float32
    PF32 = mybir.dt.float32
    ACTF = mybir.ActivationFunctionType
    AL = mybir.AluOpType

    nc = bacc.Bacc("TRN2", target_bir_lowering=False, debug=False,
                   num_devices=ndev)

    F8 = mybir.dt.float8e4
    DR = mybir.MatmulPerfMode.DoubleRow
    xtp_d = nc.dram_tensor("xtp8", [128, 8 * S], F8, kind="ExternalInput").ap()
    xq32_d = nc.dram_tensor("xq32", [D, M], F32, kind="ExternalInput").ap()
    wq_d = nc.dram_tensor("wqp8", [128, 8192], F8, kind="ExternalInput").ap()
    wk_d = nc.dram_tensor("wkp8", [128, 8192], F8, kind="ExternalInput").ap()
    wv_d = nc.dram_tensor("wvp8", [128, 8192], F8, kind="ExternalInput").ap()
    wo_d = nc.dram_tensor("wop8", [128, 8192], F8, kind="ExternalInput").ap()
    w1_d = nc.dram_tensor("w1p8", [128, 2 * DFF * (D // 256)], F8,
                          kind="ExternalInput").ap()
    cs1_d = nc.dram_tensor("cs1c", [128, DFF // 128 + 1], F32,
                           kind="ExternalInput").ap()
    w2_d = nc.dram_tensor("w2b", [DFF, D], BF, kind="ExternalInput").ap()
    mb_d = nc.dram_tensor("maskb", [128, KT], F32, kind="ExternalInput").ap()
    cv_d = nc.dram_tensor("cvec", [1, 8], F32, kind="ExternalInput").ap()
    out_d = nc.dram_tensor("outT", [D, M], mybir.dt.float32r,
                       kind="ExternalOutput").ap()

    with tile.TileContext(nc) as tc:
        with contextlib.ExitStack() as ctx:
            big = ctx.enter_context(tc.tile_pool(name="big", bufs=1))
            mid = ctx.enter_context(tc.tile_pool(name="mid", bufs=1))
            ps = ctx.enter_context(tc.tile_pool(name="ps", bufs=1, space="PSUM"))

            def big4(name):
                return big.tile([128, 2048], BF, tag="big4", bufs=16, name=name)

            def big4f(name):
                return big.tile([128, 1024], mybir.dt.float32r, tag="big4",
                                bufs=16, name=name)

            def b2k(name):
                return mid.tile([128, 1024], BF, tag="b2k", bufs=17, name=name)

            def wst8(name):
                t = mid.tile([128, 2048], F8, tag="wst", bufs=20, name=name)
                return t.rearrange("p (a b) -> p a b", a=2)

            def wst(name):
                return mid.tile([128, 1024], BF, tag="wst", bufs=20, name=name)

            def scr(shape, dt, name):
                return mid.tile(shape, dt, tag="scr", bufs=8, name=name)

            def accp(name):
                return ps.tile([128, 512], PF32, tag="acc", bufs=2, name=name)

            def scp(name):
                return ps.tile([128, 1024], PF32, tag="sc2", bufs=2, name=name)

            def emit_body():
                ones_row = mid.tile([1, 64], BF, tag="ones_r", bufs=1)
                nc.vector.memset(ones_row, 1.0)
                ones_col = mid.tile([128, 1], BF, tag="ones_c", bufs=1)
                nc.vector.memset(ones_col, 1.0)
                F32R_ = mybir.dt.float32r
                ones_f32 = mid.tile([128, 1], F32, tag="ones_f32", bufs=1)
                nc.vector.memset(ones_f32, 1.0)
                ones_rf32 = mid.tile([1, 128], F32, tag="ones_rf32", bufs=1)
                nc.vector.memset(ones_rf32, 1.0)
                ones_col32 = mid.tile([128, 1], F32R_, tag="ones_c32", bufs=1)
                nc.vector.tensor_copy(ones_col32[:, :], ones_f32[:, :])
                ones_row32 = mid.tile([1, 128], F32R_, tag="ones_r32", bufs=1)
                nc.vector.tensor_copy(ones_row32[:, :], ones_rf32[:, :])
                cvec = mid.tile([1, 8], F32, tag="cvec", bufs=1)
                nc.sync.dma_start(out=cvec, in_=cv_d)
                cs1c = mid.tile([128, DFF // 128 + 1], F32, tag="cs1c", bufs=1)
                nc.sync.dma_start(out=cs1c, in_=cs1_d)
                if mask_has_zeros:
                    mbt = mid.tile([128, KT], F32, tag="mbt", bufs=1)
                    nc.sync.dma_start(out=mbt, in_=mb_d)

                xtp = []
                for j in range(4):
                    t = big.tile([128, 2 * S], F8, tag="big4", bufs=16,
                                 name=f"xtp{j}")
                    nc.sync.dma_start(out=t,
                                      in_=xtp_d[:, j * 2 * S:(j + 1) * 2 * S])
                    xtp.append(t.rearrange("p (a b) -> p a b", a=2))

                def loadw8(w_dram, name):
                    wts = []
                    for j in range(4):
                        wt = wst8(f"{name}w{j}")
                        nc.sync.dma_start(
                            out=wt,
                            in_=w_dram[:, j * 2048:(j + 1) * 2048].rearrange(
                                "p (a b) -> p a b", a=2))
                        wts.append(wt)
                    return wts

                def proj(w_dram, n_cols, out_tiles, name):
                    wts = loadw8(w_dram, name)
                    for mo in range(KI):
                        for cp in range(n_cols // 1024):
                            acc = scp(f"{name}ps{mo}_{cp}")
                            for j in range(4):
                                for c in range(2):
                                    cc = cp * 2 + c
                                    nc.tensor.matmul(
                                        acc[:, c * 512:(c + 1) * 512],
                                        wts[j][:, :, mo * 128:(mo + 1) * 128],
                                        xtp[j][:, :, cc * 512:(cc + 1) * 512],
                                        start=(j == 0), stop=(j == 3),
                                        perf_mode=DR,
                                    )
                            with nc.allow_low_precision(reason="bf16 proj"):
                                if (mo + cp) % 2 == 0:
                                    nc.vector.tensor_copy(
                                        out_tiles[mo][:, cp * 1024:(cp + 1) * 1024],
                                        acc[:, :])
                                else:
                                    nc.scalar.copy(
                                        out=out_tiles[mo][:, cp * 1024:(cp + 1) * 1024],
                                        in_=acc[:, :])

                qt = [b2k(f"qt{i}") for i in range(KI)]
                kt = [big4(f"kt{i}") for i in range(KI)]
                if ablate == "proj":
                    for t in qt:
                        nc.vector.memset(t, 0.01)
                    for t in kt:
                        nc.vector.memset(t, 0.01)
                else:
                    proj(wq_d, M, qt, "q")
                    proj(wk_d, S, kt, "k")

                wvts = []
                if ablate != "proj":
                    wvts = loadw8(wv_d, "vw")
                vt8 = []
                for kp in range(KT // 2):
                    v = big.tile([128, 2 * H * VST], F8, tag="vt",
                                 bufs=KT // 2, name=f"vt8_{kp}")
                    vt8.append(v.rearrange("p (a b) -> p a b", a=2))
                if ablate == "proj":
                    for v in vt8:
                        nc.vector.memset(v, 0.01)
                for k in (range(KT) if ablate != "proj" else []):
                    vacc = scp(f"vps{k}")
                    for j4 in range(4):
                        for c in range(2):
                            nc.tensor.matmul(
                                vacc[:, c * 512:(c + 1) * 512],
                                xtp[j4][:, :, k * 128:(k + 1) * 128],
                                wvts[j4][:, :, c * 512:(c + 1) * 512],
                                start=(j4 == 0), stop=(j4 == 3),
                                perf_mode=DR,
                            )
                    vslot = vt8[k // 2][:, k % 2, :]
                    for c in range(2):
                        acc = vacc[:, c * 512:(c + 1) * 512]
                        dst = vslot[:, c * 8 * VST:(c * 8 + 8) * VST].rearrange(
                            "p (h j) -> p h j", j=VST)[:, :, 0:DK]
                        src = acc.rearrange("p (h j) -> p h j", j=DK)
                        with nc.allow_low_precision(reason="v stored fp8"):
                            if k % 4 < 2:
                                nc.vector.tensor_copy(dst, src)
                            else:
                                nc.scalar.copy(out=dst, in_=src)
                    ones_view = vslot.rearrange(
                        "p (h j) -> p h j", j=VST)[:, :, DK:DK + 1]
                    nc.vector.memset(ones_view, 1.0)

                wots = loadw8(wo_d, "ow")

                ctxp8 = []
                for j4 in range(4):
                    t = mid.tile([128, 2048], F8, tag="b2k", bufs=17,
                                 name=f"ctxp8_{j4}")
                    ctxp8.append(t.rearrange("p (a b) -> p a b", a=2))
                if ablate == "attn":
                    for t in ctxp8:
                        nc.vector.memset(t, 0.01)
                pending_tail = [None]
                for hp in (range(H // 2) if ablate != "attn" else []):
                    heads = (2 * hp, 2 * hp + 1)
                    cps = [
                        [ps.tile([65, 512], PF32, tag="ctxp", bufs=2,
                                 name=f"cps{hp}_{c}") for c in range(QC)],
                        [accp(f"cpsb{hp}_{c}") for c in range(QC)],
                    ]
                    et8s = {}

                    def emit_ctx(kp):
                        for hi, h in enumerate(heads):
                            for c in range(QC):
                                nc.tensor.matmul(
                                    cps[hi][c][0:65, :],
                                    vt8[kp][:, :, h * VST:(h + 1) * VST],
                                    et8s[(kp, hi)][:, :, c * 512:(c + 1) * 512],
                                    start=(kp == 0), stop=(kp == KT // 2 - 1),
                                    perf_mode=DR,
                                )
                        del et8s[(kp, 0)], et8s[(kp, 1)]

                    for k in range(KT):
                        kp, ki2 = k // 2, k % 2
                        if ki2 == 0:
                            for hi in range(2):
                                t = mid.tile([128, 2048], F8, tag="scr",
                                             bufs=8, name=f"et8_{hp}_{kp}_{hi}")
                                et8s[(kp, hi)] = t.rearrange(
                                    "p (a b) -> p a b", a=2)
                        sps = [scp(f"sps{hp}_{k}_{hi}") for hi in range(2)]
                        for hi in range(2):
                            hb = hi * 64
                            for c in range(QC):
                                nc.tensor.matmul(
                                    sps[hi][:, c * 512:(c + 1) * 512],
                                    kt[hp][hb:hb + 64, k * 128:(k + 1) * 128],
                                    qt[hp][hb:hb + 64, c * 512:(c + 1) * 512],
                                    start=True, stop=True,
                                )
                        for hi in range(2):
                            nc.scalar.activation(
                                out=et8s[(kp, hi)][:, ki2, :], in_=sps[hi][:, :],
                                func=ACTF.Exp,
                                bias=(mbt[:, k:k + 1] if mask_has_zeros
                                      else 0.0),
                                scale=0.125,
                            )
                        if k == 2 and pending_tail[0] is not None:
                            pending_tail[0]()
                            pending_tail[0] = None
                        if ki2 == 1 and kp >= 1:
                            emit_ctx(kp - 1)
                    emit_ctx(KT // 2 - 1)
                    tail_data = []
                    for hi, h in enumerate(heads):
                        cc = mid.tile([65, 1024], F32, tag="tail", bufs=2,
                                      name=f"cc{hp}_{hi}")
                        for c in range(QC):
                            nc.vector.tensor_copy(
                                cc[:, c * 512:(c + 1) * 512],
                                cps[hi][c][0:65, :])
                        den = scr([1, 1024], BF, f"den{hp}_{hi}")
                        with nc.allow_low_precision(reason="bf16 softmax denom"):
                            nc.vector.reciprocal(out=den[:, :],
                                                 in_=cc[64:65, :])
                        tail_data.append((h, cc, den))

                    def _tail(tail_data=tail_data, hp=hp):
                        for h, cc, den in tail_data:
                            bcst = [accp(f"bcst{hp}_{h}_{c}")
                                    for c in range(QC)]
                            for c in range(QC):
                                nc.tensor.matmul(
                                    bcst[c][0:64, :],
                                    ones_row[:, :],
                                    den[:, c * 512:(c + 1) * 512],
                                    start=True, stop=True)
                            for c in range(QC):
                                with nc.allow_low_precision(
                                        reason="ctx stored fp8"):
                                    nc.vector.tensor_mul(
                                        ctxp8[h // 4][
                                            64 * (h % 2):64 * (h % 2) + 64,
                                            (h % 4) // 2,
                                            c * 512:(c + 1) * 512],
                                        cc[0:64, c * 512:(c + 1) * 512],
                                        bcst[c][0:64, :])

                    pending_tail[0] = _tail
                if pending_tail[0] is not None:
                    pending_tail[0]()
                    pending_tail[0] = None

                ln1_s1 = [accp(f"ln1s1_{c}") for c in range(QC)]
                ln1_s2 = [ps.tile([65, 512], PF32, tag="ctxp", bufs=2,
                                  name=f"ln1s2_{c}") for c in range(QC)]
                trunk = []
                for mo in range(KI):
                    xq = big.tile([128, 1024], F32, tag="xq", bufs=2, name=f"xq{mo}")
                    nc.sync.dma_start(out=xq, in_=xq32_d[mo * 128:(mo + 1) * 128, :])
                    tr = big4f(f"trunk{mo}")
                    trunk.append(tr)
                    aacc = scp(f"aops{mo}")
                    for j in range(4):
                        for c in range(QC):
                            nc.tensor.matmul(
                                aacc[:, c * 512:(c + 1) * 512],
                                wots[j][:, :, mo * 128:(mo + 1) * 128],
                                ctxp8[j][:, :, c * 512:(c + 1) * 512],
                                start=(j == 0), stop=(j == 3),
                                perf_mode=DR,
                            )
                    nc.vector.tensor_add(tr[:, :], aacc[:, :], xq[:, :])
                    sq = scr([128, 1024], BF, f"ln1sq{mo}")
                    nc.scalar.activation(out=sq[:, :], in_=tr[:, :],
                                         func=ACTF.Square)
                    for c in range(QC):
                        cs = slice(c * 512, (c + 1) * 512)
                        nc.tensor.matmul(
                            ln1_s1[c][0:1, :], ones_col32[:, :], tr[:, cs],
                            start=(mo == 0), stop=(mo == KI - 1))
                        nc.tensor.matmul(
                            ln1_s2[c][0:1, :], ones_col[:, :], sq[:, cs],
                            start=(mo == 0), stop=(mo == KI - 1))

                F32R = mybir.dt.float32r

                def layer_norm(g_idx, nm, s1=None, s2=None):
                    r_s1 = big.tile([1, 1024], F32R, tag="rows", bufs=3,
                                    name=f"{nm}rs1")
                    r_tmp = big.tile([1, 1024], F32R, tag="rows", bufs=3,
                                     name=f"{nm}rtmp")
                    r_istd = big.tile([1, 1024], F32R, tag="rows", bufs=3,
                                      name=f"{nm}ristd")
                    if s1 is None:
                        s1 = [accp(f"{nm}s1_{c}") for c in range(QC)]
                        for mo in range(KI):
                            for c in range(QC):
                                nc.tensor.matmul(
                                    s1[c][0:1, :], ones_col32[:, :],
                                    trunk[mo][:, c * 512:(c + 1) * 512],
                                    start=(mo == 0), stop=(mo == KI - 1))
                    for c in range(QC):
                        cs = slice(c * 512, (c + 1) * 512)
                        nc.vector.tensor_copy(r_s1[:, cs], s1[c][0:1, :])
                    if s2 is None:
                        s2 = [accp(f"{nm}s2_{c}") for c in range(QC)]
                        for mo in range(KI):
                            sq = scr([128, 1024], BF, f"{nm}sq{mo}")
                            nc.scalar.activation(out=sq[:, :],
                                                 in_=trunk[mo][:, :],
                                                 func=ACTF.Square)
                            for c in range(QC):
                                nc.tensor.matmul(
                                    s2[c][0:1, :], ones_col[:, :],
                                    sq[:, c * 512:(c + 1) * 512],
                                    start=(mo == 0), stop=(mo == KI - 1))
                    nc.scalar.activation(out=r_tmp[:, :], in_=r_s1[:, :],
                                         func=ACTF.Square)
                    for c in range(QC):
                        cs = slice(c * 512, (c + 1) * 512)
                        nc.vector.scalar_tensor_tensor(
                            out=r_tmp[:, cs], in0=r_tmp[:, cs],
                            scalar=-1.0 / D, in1=s2[c][0:1, :],
                            op0=AL.mult, op1=AL.add)
                    nc.scalar.activation(out=r_tmp[:, :], in_=r_tmp[:, :],
                                         func=ACTF.Sqrt, scale=1.0 / (D - 1))
                    nc.vector.reciprocal(out=r_istd[:, :], in_=r_tmp[:, :])
                    nc.vector.tensor_mul(r_s1[:, :], r_s1[:, :], r_istd[:, :])
                    nc.vector.tensor_scalar(
                        out=r_s1[:, :], in0=r_s1[:, :],
                        scalar1=cvec[:, 3 * g_idx + 1:3 * g_idx + 2],
                        scalar2=cvec[:, 3 * g_idx + 2:3 * g_idx + 3],
                        op0=AL.mult, op1=AL.add)
                    nc.vector.tensor_scalar_mul(
                        out=r_istd[:, :], in0=r_istd[:, :],
                        scalar1=cvec[:, 3 * g_idx:3 * g_idx + 1])
                    for c in range(QC):
                        cs = slice(c * 512, (c + 1) * 512)
                        bc = scp(f"{nm}bc{c}")
                        nc.tensor.matmul(
                            bc[:, 0:512], ones_row32[:, :],
                            r_istd[:, cs],
                            start=True, stop=True)
                        nc.tensor.matmul(
                            bc[:, 512:1024], ones_row32[:, :],
                            r_s1[:, cs],
                            start=True, stop=True)
                        for mo in range(KI):
                            nc.vector.tensor_mul(trunk[mo][:, cs],
                                                 trunk[mo][:, cs], bc[:, 0:512])
                            nc.vector.tensor_add(trunk[mo][:, cs],
                                                 trunk[mo][:, cs], bc[:, 512:1024])

                layer_norm(0, "ln1", s1=ln1_s1, s2=ln1_s2)

                KP = D // 256
                x2f8 = []
                for kp in range(KP):
                    t = mid.tile([128, 2048], F8, tag="b2k", bufs=17,
                                 name=f"x2f8_{kp}")
                    x2f8.append(t.rearrange("p (a b) -> p a b", a=2))
                for mo in range(KI):
                    with nc.allow_low_precision(reason="ffn input fp8"):
                        dst = x2f8[mo // 2][:, mo % 2, :]
                        if mo % 2 == 0:
                            nc.scalar.activation(
                                out=dst, in_=trunk[mo][:, :],
                                func=ACTF.Identity,
                                bias=cs1c[:, 32:33], scale=1.0)
                        else:
                            nc.vector.tensor_scalar(
                                out=dst, in0=trunk[mo][:, :],
                                scalar1=cs1c[:, 32:33], scalar2=None,
                                op0=AL.add)

                w1_v = w1_d.rearrange("p (a f) -> p a f", a=2 * KP)
                for g in (range(4) if ablate != "ffn" else []):
                    w1ts = []
                    for kp in range(KP):
                        wt = mid.tile([128, 2048], F8, tag="wst", bufs=20,
                                      name=f"w1t{g}_{kp}")
                        nc.sync.dma_start(
                            out=wt.rearrange("p (a b) -> p a b", a=2),
                            in_=w1_v[:, 2 * kp:2 * kp + 2,
                                     g * 1024:(g + 1) * 1024])
                        w1ts.append(wt.rearrange("p (a b) -> p a b", a=2))
                    ffb = []
                    for fl in range(8):
                        fb = scr([128, 1024], BF, f"ffb{g}_{fl}")
                        ffb.append(fb)
                        acc = scp(f"f1ps{g}_{fl}")
                        for kp in range(KP):
                            for c in range(QC):
                                nc.tensor.matmul(
                                    acc[:, c * 512:(c + 1) * 512],
                                    w1ts[kp][:, :, fl * 128:(fl + 1) * 128],
                                    x2f8[kp][:, :, c * 512:(c + 1) * 512],
                                    start=(kp == 0), stop=(kp == KP - 1),
                                    perf_mode=DR,
                                )
                        nc.scalar.activation(
                            out=fb[:, :], in_=acc[:, :], func=ACTF.Relu,
                            bias=cs1c[:, g * 8 + fl:g * 8 + fl + 1],
                            scale=1.0 / 16.0)
                    w2ts = []
                    for fl in range(8):
                        wt = wst(f"w2t{g}_{fl}")
                        nc.sync.dma_start(
                            out=wt,
                            in_=w2_d[(g * 8 + fl) * 128:(g * 8 + fl + 1) * 128, :])
                        w2ts.append(wt)
                    for mo in range(KI):
                        acc = [accp(f"f2ps{g}_{mo}_{c}") for c in range(QC)]
                        for fl in range(8):
                            for c in range(QC):
                                nc.tensor.matmul(
                                    acc[c][:, :],
                                    w2ts[fl][:, mo * 128:(mo + 1) * 128],
                                    ffb[fl][:, c * 512:(c + 1) * 512],
                                    start=(fl == 0), stop=(fl == 7),
                                )
                        for c in range(QC):
                            cs = slice(c * 512, (c + 1) * 512)
                            nc.vector.tensor_add(trunk[mo][:, cs],
                                                 trunk[mo][:, cs], acc[c][:, :])

                layer_norm(1, "ln2")

                for mo in range(KI):
                    nc.sync.dma_start(out=out_d[mo * 128:(mo + 1) * 128, :],
                                      in_=trunk[mo][:, :])

            for _rep in range(reps):
                with nc.allow_low_precision(
                        reason="f32r trunk: fp22 write rounding, verified "
                               "against fp32 reference"):
                    emit_body()

    nc.compile()
    return nc


def build2(mask_has_zeros: bool, reps: int = 1, ndev=N_CORES):
    import concourse.bass as bass
    import concourse.mybir as mybir
    import concourse.tile as tile
    from concourse import bacc
    import contextlib

    BF = mybir.dt.bfloat16
    F32 = mybir.dt.float32
    PF32 = mybir.dt.float32
    F32R = mybir.dt.float32r
    ACTF = mybir.ActivationFunctionType
    AL = mybir.AluOpType

    nc = bacc.Bacc("TRN2", target_bir_lowering=False, debug=False,
                   num_devices=ndev)

    F8 = mybir.dt.float8e4
    DR = mybir.MatmulPerfMode.DoubleRow
    xtp_d = nc.dram_tensor("xtp8", [128, 8 * S], F8, kind="ExternalInput").ap()
    xq32_d = nc.dram_tensor("xq32", [D, M], F32, kind="ExternalInput").ap()
    wq_d = nc.dram_tensor("wqp8", [128, 8192], F8, kind="ExternalInput").ap()
    wk_d = nc.dram_tensor("wkp8", [128, 8192], F8, kind="ExternalInput").ap()
    wv_d = nc.dram_tensor("wvp8", [128, 8192], F8, kind="ExternalInput").ap()
    wo_d = nc.dram_tensor("wop8", [128, 8192], F8, kind="ExternalInput").ap()
    w1_d = nc.dram_tensor("w1p8", [128, 2 * DFF * (D // 256)], F8,
                          kind="ExternalInput").ap()
    cs1_d = nc.dram_tensor("cs1c", [128, DFF // 128 + 1], F32,
                           kind="ExternalInput").ap()
    w2_d = nc.dram_tensor("w2b", [DFF, D], BF, kind="ExternalInput").ap()
    mb_d = nc.dram_tensor("maskb", [128, KT], F32, kind="ExternalInput").ap()
    cv_d = nc.dram_tensor("cvec", [1, 8], F32, kind="ExternalInput").ap()
    out_d = nc.dram_tensor("outT", [D, M], F32R, kind="ExternalOutput").ap()

    HM = M // 2
    KP = D // 256

    with tile.TileContext(nc) as tc:
        with contextlib.ExitStack() as ctx:
            big = ctx.enter_context(tc.tile_pool(name="big", bufs=1))
            mid = ctx.enter_context(tc.tile_pool(name="mid", bufs=1))
            ps = ctx.enter_context(tc.tile_pool(name="ps", bufs=1, space="PSUM"))

            def big4(name):
                return big.tile([128, 2048], BF, tag="big4", bufs=12, name=name)

            def trk(name):
                return big.tile([128, 512], F32R, tag="trk", bufs=12, name=name)

            def b2k(name):
                return mid.tile([128, 1024], BF, tag="b2k", bufs=8, name=name)

            def wst8(name):
                t = mid.tile([128, 2048], F8, tag="wst", bufs=12, name=name)
                return t.rearrange("p (a b) -> p a b", a=2)

            def wst(name):
                return mid.tile([128, 1024], BF, tag="wst", bufs=12, name=name)

            def scp(name):
                return ps.tile([128, 1024], PF32, tag="sc", bufs=2, name=name)

            def ctxp(rows, name):
                return ps.tile([rows, 512], PF32, tag="ctx", bufs=2, name=name)

            def trp(name):
                return ps.tile([128, 512], PF32, tag="tr", bufs=2, name=name)

            def emit_all():
                ones_row = mid.tile([1, 64], BF, tag="ones_r", bufs=1)
                nc.vector.memset(ones_row, 1.0)
                ones_col = mid.tile([128, 1], BF, tag="ones_c", bufs=1)
                nc.vector.memset(ones_col, 1.0)
                ones_f32 = mid.tile([128, 1], F32, tag="ones_f32", bufs=1)
                nc.vector.memset(ones_f32, 1.0)
                ones_rf32 = mid.tile([1, 128], F32, tag="ones_rf32", bufs=1)
                nc.vector.memset(ones_rf32, 1.0)
                ones_col32 = mid.tile([128, 1], F32R, tag="ones_c32", bufs=1)
                nc.vector.tensor_copy(ones_col32[:, :], ones_f32[:, :])
                ones_row32 = mid.tile([1, 128], F32R, tag="ones_r32", bufs=1)
                nc.vector.tensor_copy(ones_row32[:, :], ones_rf32[:, :])
                cvec = mid.tile([1, 8], F32, tag="cvec", bufs=1)
                nc.sync.dma_start(out=cvec, in_=cv_d)
                cs1c = mid.tile([128, DFF // 128 + 1], F32, tag="cs1c", bufs=1)
                nc.sync.dma_start(out=cs1c, in_=cs1_d)
                mbt = None
                if mask_has_zeros:
                    mbt = mid.tile([128, KT], F32, tag="mbt", bufs=1)
                    nc.sync.dma_start(out=mbt, in_=mb_d)

                def loadw8(w_dram, name, tag="wst", bufs=12):
                    wts = []
                    for j in range(4):
                        t = mid.tile([128, 2048], F8, tag=tag, bufs=bufs,
                                     name=f"{name}w{j}")
                        wt = t.rearrange("p (a b) -> p a b", a=2)
                        nc.sync.dma_start(
                            out=wt,
                            in_=w_dram[:, j * 2048:(j + 1) * 2048].rearrange(
                                "p (a b) -> p a b", a=2))
                        wts.append(wt)
                    return wts

                def emit_phase0(r):
                    xtp = []
                    for j in range(4):
                        t = big.tile([128, 2 * S], F8, tag="big4", bufs=12,
                                     name=f"xtp{j}_{r}")
                        nc.sync.dma_start(
                            out=t, in_=xtp_d[:, j * 2 * S:(j + 1) * 2 * S])
                        xtp.append(t.rearrange("p (a b) -> p a b", a=2))

                    def proj(w_dram, n_cols, out_tiles, name):
                        wts = loadw8(w_dram, name)
                        for mo in range(KI):
                            for cp in range(n_cols // 1024):
                                acc = scp(f"{name}ps{mo}_{cp}_{r}")
                                for j in range(4):
                                    for c in range(2):
                                        cc = cp * 2 + c
                                        nc.tensor.matmul(
                                            acc[:, c * 512:(c + 1) * 512],
                                            wts[j][:, :, mo * 128:(mo + 1) * 128],
                                            xtp[j][:, :, cc * 512:(cc + 1) * 512],
                                            start=(j == 0), stop=(j == 3),
                                            perf_mode=DR,
                                        )
                                with nc.allow_low_precision(reason="bf16 proj"):
                                    if (mo + cp) % 2 == 0:
                                        nc.vector.tensor_copy(
                                            out_tiles[mo][:, cp * 1024:(cp + 1) * 1024],
                                            acc[:, :])
                                    else:
                                        nc.scalar.copy(
                                            out=out_tiles[mo][:, cp * 1024:(cp + 1) * 1024],
                                            in_=acc[:, :])

                    qt = [b2k(f"qt{i}_{r}") for i in range(KI)]
                    kt = [big4(f"kt{i}_{r}") for i in range(KI)]
                    proj(wq_d, M, qt, "q")
                    proj(wk_d, S, kt, "k")

                    wvts = loadw8(wv_d, "vw")
                    vt8 = []
                    for kp in range(KT // 2):
                        v = big.tile([128, 2 * H * VST], F8, tag="vt",
                                     bufs=KT // 2, name=f"vt8_{kp}_{r}")
                        vt8.append(v.rearrange("p (a b) -> p a b", a=2))
                    for k in range(KT):
                        vacc = scp(f"vps{k}_{r}")
                        for j4 in range(4):
                            for c in range(2):
                                nc.tensor.matmul(
                                    vacc[:, c * 512:(c + 1) * 512],
                                    xtp[j4][:, :, k * 128:(k + 1) * 128],
                                    wvts[j4][:, :, c * 512:(c + 1) * 512],
                                    start=(j4 == 0), stop=(j4 == 3),
                                    perf_mode=DR,
                                )
                        vslot = vt8[k // 2][:, k % 2, :]
                        for c in range(2):
                            acc = vacc[:, c * 512:(c + 1) * 512]
                            dst = vslot[:, c * 8 * VST:(c * 8 + 8) * VST].rearrange(
                                "p (h j) -> p h j", j=VST)[:, :, 0:DK]
                            src = acc.rearrange("p (h j) -> p h j", j=DK)
                            with nc.allow_low_precision(reason="v stored fp8"):
                                if k % 4 < 2:
                                    nc.vector.tensor_copy(dst, src)
                                else:
                                    nc.scalar.copy(out=dst, in_=src)
                        ones_view = vslot.rearrange(
                            "p (h j) -> p h j", j=VST)[:, :, DK:DK + 1]
                        nc.vector.memset(ones_view, 1.0)

                    wots = loadw8(wo_d, "ow", tag="wot", bufs=8)
                    ctxp8 = []
                    for j4 in range(4):
                        t = mid.tile([128, 2048], F8, tag="cx8", bufs=8,
                                     name=f"ctxp8_{j4}_{r}")
                        ctxp8.append(t.rearrange("p (a b) -> p a b", a=2))
                    return dict(qt=qt, kt=kt, vt8=vt8, wots=wots,
                                ctxp8=ctxp8, cvec=cvec, cs1c=cs1c, mbt=mbt)

                def attn_half(st, h, r):
                    qt, kt, vt8 = st["qt"], st["kt"], st["vt8"]
                    ctxp8, mbt = st["ctxp8"], st["mbt"]
                    hs = slice(h * HM, (h + 1) * HM)
                    for hp in range(H // 2):
                        heads = (2 * hp, 2 * hp + 1)
                        cps = [ctxp(65 if hi == 0 else 128,
                                    f"cps{hp}_{hi}_{h}_{r}")
                               for hi in range(2)]
                        et8s = {}

                        def emit_ctx(kp):
                            for hi, hd in enumerate(heads):
                                nc.tensor.matmul(
                                    cps[hi][0:65, :],
                                    vt8[kp][:, :, hd * VST:(hd + 1) * VST],
                                    et8s[(kp, hi)],
                                    start=(kp == 0), stop=(kp == KT // 2 - 1),
                                    perf_mode=DR,
                                )
                            del et8s[(kp, 0)], et8s[(kp, 1)]

                        for kp in range(KT // 2):
                            sps = [scp(f"sps{hp}_{kp}_{hi}_{h}_{r}")
                                   for hi in range(2)]
                            for ki2 in range(2):
                                k = 2 * kp + ki2
                                for hi in range(2):
                                    hb = hi * 64
                                    nc.tensor.matmul(
                                        sps[hi][:, ki2 * 512:(ki2 + 1) * 512],
                                        kt[hp][hb:hb + 64, k * 128:(k + 1) * 128],
                                        qt[hp][hb:hb + 64, hs],
                                        start=True, stop=True,
                                    )
                            for hi in range(2):
                                t = mid.tile([128, 1024], F8, tag="et8",
                                             bufs=6, name=f"et8_{hp}_{kp}_{hi}_{h}_{r}")
                                if mask_has_zeros:
                                    for ki2 in range(2):
                                        nc.scalar.activation(
                                            out=t[:, ki2 * 512:(ki2 + 1) * 512],
                                            in_=sps[hi][:, ki2 * 512:(ki2 + 1) * 512],
                                            func=ACTF.Exp,
                                            bias=mbt[:, 2 * kp + ki2:2 * kp + ki2 + 1],
                                            scale=0.125)
                                else:
                                    nc.scalar.activation(
                                        out=t[:, :], in_=sps[hi][:, :],
                                        func=ACTF.Exp, bias=0.0, scale=0.125)
                                et8s[(kp, hi)] = t.rearrange(
                                    "p (a b) -> p a b", a=2)
                            if kp >= 1:
                                emit_ctx(kp - 1)
                        emit_ctx(KT // 2 - 1)
                        for hi, hd in enumerate(heads):
                            cc = mid.tile([65, 512], F32, tag="tail", bufs=4,
                                          name=f"cc{hp}_{hi}_{h}_{r}")
                            nc.vector.tensor_copy(cc[:, :], cps[hi][0:65, :])
                            den = mid.tile([1, 512], BF, tag="den", bufs=2,
                                           name=f"den{hp}_{hi}_{h}_{r}")
                            with nc.allow_low_precision(reason="bf16 denom"):
                                nc.vector.reciprocal(out=den[:, :],
                                                     in_=cc[64:65, :])
                            bc = ctxp(64, f"bcst{hp}_{hi}_{h}_{r}")
                            nc.tensor.matmul(bc[0:64, :], ones_row[:, :],
                                             den[:, :], start=True, stop=True)
                            with nc.allow_low_precision(reason="ctx fp8"):
                                nc.vector.tensor_mul(
                                    ctxp8[hd // 4][
                                        64 * (hd % 2):64 * (hd % 2) + 64,
                                        (hd % 4) // 2, hs],
                                    cc[0:64, :], bc[0:64, :])
                        yield

                def trunk_half(st, h, r):
                    wots, ctxp8 = st["wots"], st["ctxp8"]
                    cvec, cs1c = st["cvec"], st["cs1c"]
                    hs = slice(h * HM, (h + 1) * HM)
                    trunk = []
                    for mo in range(KI):
                        xq = mid.tile([128, 512], F32, tag="xq", bufs=2,
                                      name=f"xq{mo}_{h}_{r}")
                        nc.sync.dma_start(
                            out=xq, in_=xq32_d[mo * 128:(mo + 1) * 128, hs])
                        tr = trk(f"trunk{mo}_{h}_{r}")
                        trunk.append(tr)
                        aacc = trp(f"aops{mo}_{h}_{r}")
                        for j in range(4):
                            nc.tensor.matmul(
                                aacc[:, :],
                                wots[j][:, :, mo * 128:(mo + 1) * 128],
                                ctxp8[j][:, :, hs],
                                start=(j == 0), stop=(j == 3),
                                perf_mode=DR,
                            )
                        nc.vector.tensor_add(tr[:, :], aacc[:, :], xq[:, :])
                        if mo == 3:
                            yield
                    yield

                    def layer_norm(g_idx, nm):
                        s1 = trp(f"{nm}s1_{h}_{r}")
                        s2 = trp(f"{nm}s2_{h}_{r}")
                        for mo in range(KI):
                            nc.tensor.matmul(
                                s1[0:1, :], ones_col32[:, :], trunk[mo][:, :],
                                start=(mo == 0), stop=(mo == KI - 1))
                        for mo in range(KI):
                            sq = mid.tile([128, 512], BF, tag="sqr", bufs=2,
                                          name=f"{nm}sq{mo}_{h}_{r}")
                            nc.scalar.activation(out=sq[:, :],
                                                 in_=trunk[mo][:, :],
                                                 func=ACTF.Square)
                            nc.tensor.matmul(
                                s2[0:1, :], ones_col[:, :], sq[:, :],
                                start=(mo == 0), stop=(mo == KI - 1))
                        r_s1 = mid.tile([1, 512], F32R, tag="rows", bufs=4,
                                        name=f"{nm}rs1_{h}_{r}")
                        r_tmp = mid.tile([1, 512], F32R, tag="rows", bufs=4,
                                         name=f"{nm}rtmp_{h}_{r}")
                        r_istd = mid.tile([1, 512], F32R, tag="rows", bufs=4,
                                          name=f"{nm}ristd_{h}_{r}")
                        nc.vector.tensor_copy(r_s1[:, :], s1[0:1, :])
                        nc.scalar.activation(out=r_tmp[:, :], in_=r_s1[:, :],
                                             func=ACTF.Square)
                        nc.vector.scalar_tensor_tensor(
                            out=r_tmp[:, :], in0=r_tmp[:, :],
                            scalar=-1.0 / D, in1=s2[0:1, :],
                            op0=AL.mult, op1=AL.add)
                        nc.scalar.activation(out=r_tmp[:, :], in_=r_tmp[:, :],
                                             func=ACTF.Sqrt, scale=1.0 / (D - 1))
                        nc.vector.reciprocal(out=r_istd[:, :], in_=r_tmp[:, :])
                        nc.vector.tensor_mul(r_s1[:, :], r_s1[:, :], r_istd[:, :])
                        nc.vector.tensor_scalar(
                            out=r_s1[:, :], in0=r_s1[:, :],
                            scalar1=cvec[:, 3 * g_idx + 1:3 * g_idx + 2],
                            scalar2=cvec[:, 3 * g_idx + 2:3 * g_idx + 3],
                            op0=AL.mult, op1=AL.add)
                        nc.vector.tensor_scalar_mul(
                            out=r_istd[:, :], in0=r_istd[:, :],
                            scalar1=cvec[:, 3 * g_idx:3 * g_idx + 1])
                        bca = trp(f"{nm}bca_{h}_{r}")
                        bcb = trp(f"{nm}bcb_{h}_{r}")
                        nc.tensor.matmul(bca[:, :], ones_row32[:, :],
                                         r_istd[:, :], start=True, stop=True)
                        nc.tensor.matmul(bcb[:, :], ones_row32[:, :],
                                         r_s1[:, :], start=True, stop=True)
                        for mo in range(KI):
                            nc.vector.tensor_mul(trunk[mo][:, :],
                                                 trunk[mo][:, :], bca[:, :])
                            nc.vector.tensor_add(trunk[mo][:, :],
                                                 trunk[mo][:, :], bcb[:, :])

                    layer_norm(0, "ln1")
                    yield

                    x2f8 = []
                    for kp in range(KP):
                        t = mid.tile([128, 1024], F8, tag="x28", bufs=8,
                                     name=f"x2f8_{kp}_{h}_{r}")
                        x2f8.append(t.rearrange("p (a b) -> p a b", a=2))
                    for mo in range(KI):
                        with nc.allow_low_precision(reason="ffn input fp8"):
                            dst = x2f8[mo // 2][:, mo % 2, :]
                            if mo % 2 == 0:
                                nc.scalar.activation(
                                    out=dst, in_=trunk[mo][:, :],
                                    func=ACTF.Identity,
                                    bias=cs1c[:, 32:33], scale=1.0)
                            else:
                                nc.vector.tensor_scalar(
                                    out=dst, in0=trunk[mo][:, :],
                                    scalar1=cs1c[:, 32:33], scalar2=None,
                                    op0=AL.add)
                    yield

                    w1_v = w1_d.rearrange("p (a f) -> p a f", a=2 * KP)
                    for g in range(4):
                        w1ts = []
                        for kp in range(KP):
                            wt = mid.tile([128, 2048], F8, tag="wst", bufs=12,
                                          name=f"w1t{g}_{kp}_{h}_{r}")
                            nc.sync.dma_start(
                                out=wt.rearrange("p (a b) -> p a b", a=2),
                                in_=w1_v[:, 2 * kp:2 * kp + 2,
                                         g * 1024:(g + 1) * 1024])
                            w1ts.append(wt.rearrange("p (a b) -> p a b", a=2))
                        ffb = []
                        for fl in range(8):
                            fb = mid.tile([128, 512], BF, tag="ffb", bufs=9,
                                          name=f"ffb{g}_{fl}_{h}_{r}")
                            ffb.append(fb)
                            acc = trp(f"f1ps{g}_{fl}_{h}_{r}")
                            for kp in range(KP):
                                nc.tensor.matmul(
                                    acc[:, :],
                                    w1ts[kp][:, :, fl * 128:(fl + 1) * 128],
                                    x2f8[kp][:, :, :],
                                    start=(kp == 0), stop=(kp == KP - 1),
                                    perf_mode=DR,
                                )
                            nc.scalar.activation(
                                out=fb[:, :], in_=acc[:, :], func=ACTF.Relu,
                                bias=cs1c[:, g * 8 + fl:g * 8 + fl + 1],
                                scale=1.0 / 16.0)
                        w2ts = []
                        for fl in range(8):
                            wt = wst(f"w2t{g}_{fl}_{h}_{r}")
                            nc.sync.dma_start(
                                out=wt,
                                in_=w2_d[(g * 8 + fl) * 128:(g * 8 + fl + 1) * 128, :])
                            w2ts.append(wt)
                        for mo in range(KI):
                            acc = trp(f"f2ps{g}_{mo}_{h}_{r}")
                            for fl in range(8):
                                nc.tensor.matmul(
                                    acc[:, :],
                                    w2ts[fl][:, mo * 128:(mo + 1) * 128],
                                    ffb[fl][:, :],
                                    start=(fl == 0), stop=(fl == 7),
                                )
                            nc.vector.tensor_add(trunk[mo][:, :],
                                                 trunk[mo][:, :], acc[:, :])
                        yield

                    layer_norm(1, "ln2")
                    for mo in range(KI):
                        nc.sync.dma_start(
                            out=out_d[mo * 128:(mo + 1) * 128, hs],
                            in_=trunk[mo][:, :])
                    yield

                def interleave(gen_a, gen_t):
                    done_a = done_t = False
                    while not (done_a and done_t):
                        if not done_t:
                            done_t = next(gen_t, _SENTINEL) is _SENTINEL
                        if not done_a:
                            done_a = next(gen_a, _SENTINEL) is _SENTINEL

                pending = None
                for r in range(reps):
                    st = emit_phase0(r)
                    a0 = attn_half(st, 0, r)
                    if pending is not None:
                        interleave(a0, pending)
                    else:
                        for _ in a0:
                            pass
                    interleave(attn_half(st, 1, r), trunk_half(st, 0, r))
                    pending = trunk_half(st, 1, r)
                for _ in pending:
                    pass

            with nc.allow_low_precision(
                    reason="f32r trunk: fp22 write rounding, verified "
                           "against fp32 reference"):
                emit_all()

    nc.compile()
    return nc


_SENTINEL = object()


_NC_CACHE = {}


def _get_nc(mask_has_zeros: bool):
    if mask_has_zeros not in _NC_CACHE:
        _NC_CACHE[mask_has_zeros] = build2(mask_has_zeros)
    return _NC_CACHE[mask_has_zeros]


def _reference_numpy(x, mask, wq, bq, wk, bk, wv, bv, wo, bo, w1, b1, w2, b2,
                     g1, bt1, g2, bt2):
    import math
    out = np.zeros_like(x)

    def ln(v, g, bt):
        mean = v.mean(-1, keepdims=True)
        std = v.std(-1, keepdims=True, ddof=1)
        return g * ((v - mean) / std + EPS) + bt

    for b in range(B):
        xb = x[b]
        q = (xb @ wq + bq).reshape(S, H, DK).transpose(1, 0, 2)
        k = (xb @ wk + bk).reshape(S, H, DK).transpose(1, 0, 2)
        v = (xb @ wv + bv).reshape(S, H, DK).transpose(1, 0, 2)
        ctx = np.zeros((H, S, DK), np.float32)
        mrow = mask[b, 0, 0, :]
        for h in range(H):
            sc = (q[h] @ k[h].T) / math.sqrt(DK)
            sc = np.where(mrow[None, :] == 0, np.float32(-1e9), sc)
            e = np.exp(sc - sc.max(-1, keepdims=True))
            p = e / e.sum(-1, keepdims=True)
            ctx[h] = p @ v[h]
        cx = ctx.transpose(1, 0, 2).reshape(S, D)
        x1 = ln(xb + cx @ wo + bo, g1, bt1)
        ff = np.maximum(x1 @ w1 + b1, 0.0) @ w2 + b2
        out[b] = ln(x1 + ff, g2, bt2)
    return out


def kernel(**inputs) -> np.ndarray:
    from concourse.bass_utils import run_bass_kernel_spmd

    x = np.asarray(inputs["x"], np.float32)
    mask = np.asarray(inputs["mask"])
    wq, wk, wv, wo = (np.asarray(inputs[k], np.float32)
                      for k in ("wq", "wk", "wv", "wo"))
    w1 = np.asarray(inputs["w1"], np.float32)
    w2 = np.asarray(inputs["w2"], np.float32)
    g1 = float(np.asarray(inputs["g1"]))
    bt1 = float(np.asarray(inputs["bt1"]))
    g2 = float(np.asarray(inputs["g2"]))
    bt2 = float(np.asarray(inputs["bt2"]))
    biases = [np.asarray(inputs[k], np.float32)
              for k in ("bq", "bk", "bv", "bo", "b1", "b2")]

    if any(np.abs(b).max() > 0 for b in biases):
        return _reference_numpy(
            x, mask, wq, biases[0], wk, biases[1], wv, biases[2], wo,
            biases[3], w1, biases[4], w2, biases[5], g1, bt1, g2, bt2)

    mask_has_zeros = bool((mask == 0).any())
    nc = _get_nc(mask_has_zeros)
    in_maps = _prepare_in_maps(x, mask, wq, wk, wv, wo, w1, w2,
                               g1, bt1, g2, bt2)

    res = run_bass_kernel_spmd(nc, in_maps, core_ids=list(range(N_CORES)))
    globals()["LAST_RESULTS"] = res

    out = np.empty((B, S, D), np.float32)
    for core in range(N_CORES):
        b = core // 2
        qoff = (core % 2) * M
        out[b, qoff:qoff + M, :] = res.results[core]["outT"].T
    return out


F8NP = ml_dtypes.float8_e4m3


def _pair_rows(w):
    K_, F_ = w.shape
    t = w.reshape(K_ // 256, 2, 128, F_).transpose(2, 0, 1, 3)
    return np.ascontiguousarray(t.reshape(128, -1)).astype(F8NP)


def _prepare_in_maps(x, mask, wq, wk, wv, wo, w1, w2, g1, bt1, g2, bt2):
    wqp = _pair_rows(wq)
    wkp = _pair_rows(wk)
    wvp = _pair_rows(wv)
    wop = _pair_rows(wo)
    w1p = _pair_rows(w1 * 16.0)
    c1 = g1 * EPS + bt1
    cs1c = np.concatenate(
        [(c1 * w1.sum(0, dtype=np.float64)).astype(np.float32)
         .reshape(DFF // 128, 128).T,
         np.full((128, 1), -c1, np.float32)], axis=1)
    cs1c = np.ascontiguousarray(cs1c)
    w2b = w2.astype(NB)
    cvec = np.array([[g1, -g1 / D, g1 * EPS + bt1,
                      g2, -g2 / D, g2 * EPS + bt2, 0.0, 0.0]], np.float32)

    in_maps = []
    for core in range(N_CORES):
        b = core // 2
        qoff = (core % 2) * M
        xT = np.ascontiguousarray(x[b].T)
        mrow = np.where(mask[b, 0, 0, :] == 0, np.float32(-1e9),
                        np.float32(0.0)).astype(np.float32)
        if qoff:
            xT_k = np.concatenate([xT[:, M:], xT[:, :M]], axis=1)
            mrow = np.concatenate([mrow[M:], mrow[:M]])
        else:
            xT_k = xT
        in_maps.append({
            "xtp8": _pair_rows(xT_k),
            "xq32": np.ascontiguousarray(xT[:, qoff:qoff + M]),
            "wqp8": wqp, "wkp8": wkp, "wvp8": wvp, "wop8": wop,
            "w1p8": w1p, "cs1c": cs1c, "w2b": w2b,
            "maskb": np.ascontiguousarray(mrow.reshape(KT, 128).T),
            "cvec": cvec,
        })
    return in_maps


if __name__ == "__main__":
    d = np.load("/root/problem/ref_cache.npz")
    inputs = {k: d[k] for k in d.files if k != "exp"}
    got = kernel(**inputs)
    exp = d["exp"]
    err = np.abs(got - exp)
    print("max abs err:", err.max())
    print("rel max:", err.max() / np.abs(exp).max())
    print("rel l2:", np.linalg.norm(err) / np.linalg.norm(exp))



# revision 12
# speedup vs baseline: 10.3048x; 10.3048x over previous
import sys
import numpy as np

for _p in ("/root/.axon_site", "/root/.axon_site/_ro/trn_rl_repo",
           "/root/.axon_site/_ro/pypackages", "/opt/trn_rl_repo"):
    if _p not in sys.path:
        sys.path.append(_p)

import ml_dtypes

B, S, D, H, DFF = 4, 2048, 1024, 16, 4096
DK = D // H
EPS = 1e-9
N_CORES = 8
M = S // 2
NB = ml_dtypes.bfloat16

KI = D // 128
KT = S // 128
QC = M // 512
VST = DK + 1


def build(mask_has_zeros: bool, reps: int = 1, ablate=None, ndev=N_CORES):
    import concourse.bass as bass
    import concourse.mybir as mybir
    import concourse.tile as tile
    from concourse import bacc
    import contextlib

    BF = mybir.dt.bfloat16
    F32 = mybir.dt.float32
    PF32 = mybir.dt.float32
    ACTF = mybir.ActivationFunctionType
    AL = mybir.AluOpType

    nc = bacc.Bacc("TRN2", target_bir_lowering=False, debug=False,
                   num_devices=ndev)

    F8 = mybir.dt.float8e4
    DR = mybir.MatmulPerfMode.DoubleRow
    xtp_d = nc.dram_tensor("xtp8", [128, 8 * S], F8, kind="ExternalInput").ap()
    xq32_d = nc.dram_tensor("xq32", [D, M], F32, kind="ExternalInput").ap()
    wq_d = nc.dram_tensor("wqp8", [128, 8192], F8, kind="ExternalInput").ap()
    wk_d = nc.dram_tensor("wkp8", [128, 8192], F8, kind="ExternalInput").ap()
    wv_d = nc.dram_tensor("wvp8", [128, 8192], F8, kind="ExternalInput").ap()
    wo_d = nc.dram_tensor("wop8", [128, 8192], F8, kind="ExternalInput").ap()
    w1_d = nc.dram_tensor("w1p8", [128, 2 * DFF * (D // 256)], F8,
                          kind="ExternalInput").ap()
    cs1_d = nc.dram_tensor("cs1c", [128, DFF // 128 + 1], F32,
                           kind="ExternalInput").ap()
    w2_d = nc.dram_tensor("w2b", [DFF, D], BF, kind="ExternalInput").ap()
    mb_d = nc.dram_tensor("maskb", [128, KT], F32, kind="ExternalInput").ap()
    cv_d = nc.dram_tensor("cvec", [1, 8], F32, kind="ExternalInput").ap()
    out_d = nc.dram_tensor("outT", [D, M], mybir.dt.float32r,
                       kind="ExternalOutput").ap()

    with tile.TileContext(nc) as tc:
        with contextlib.ExitStack() as ctx:
            big = ctx.enter_context(tc.tile_pool(name="big", bufs=1))
            mid = ctx.enter_context(tc.tile_pool(name="mid", bufs=1))
            ps = ctx.enter_context(tc.tile_pool(name="ps", bufs=1, space="PSUM"))

            def big4(name):
                return big.tile([128, 2048], BF, tag="big4", bufs=16, name=name)

            def big4f(name):
                return big.tile([128, 1024], mybir.dt.float32r, tag="big4",
                                bufs=16, name=name)

            def b2k(name):
                return mid.tile([128, 1024], BF, tag="b2k", bufs=17, name=name)

            def wst8(name):
                t = mid.tile([128, 2048], F8, tag="wst", bufs=20, name=name)
                return t.rearrange("p (a b) -> p a b", a=2)

            def wst(name):
                return mid.tile([128, 1024], BF, tag="wst", bufs=20, name=name)

            def scr(shape, dt, name):
                return mid.tile(shape, dt, tag="scr", bufs=8, name=name)

            def accp(name):
                return ps.tile([128, 512], PF32, tag="acc", bufs=2, name=name)

            def scp(name):
                return ps.tile([128, 1024], PF32, tag="sc2", bufs=2, name=name)

            def emit_body():
                ones_row = mid.tile([1, 64], BF, tag="ones_r", bufs=1)
                nc.vector.memset(ones_row, 1.0)
                ones_col = mid.tile([128, 1], BF, tag="ones_c", bufs=1)
                nc.vector.memset(ones_col, 1.0)
                F32R_ = mybir.dt.float32r
                ones_f32 = mid.tile([128, 1], F32, tag="ones_f32", bufs=1)
                nc.vector.memset(ones_f32, 1.0)
                ones_rf32 = mid.tile([1, 128], F32, tag="ones_rf32", bufs=1)
                nc.vector.memset(ones_rf32, 1.0)
                ones_col32 = mid.tile([128, 1], F32R_, tag="ones_c32", bufs=1)
                nc.vector.tensor_copy(ones_col32[:, :], ones_f32[:, :])
                ones_row32 = mid.tile([1, 128], F32R_, tag="ones_r32", bufs=1)
                nc.vector.tensor_copy(ones_row32[:, :], ones_rf32[:, :])
                cvec = mid.tile([1, 8], F32, tag="cvec", bufs=1)
                nc.sync.dma_start(out=cvec, in_=cv_d)
                cs1c = mid.tile([128, DFF // 128 + 1], F32, tag="cs1c", bufs=1)
                nc.sync.dma_start(out=cs1c, in_=cs1_d)
                if mask_has_zeros:
                    mbt = mid.tile([128, KT], F32, tag="mbt", bufs=1)
                    nc.sync.dma_start(out=mbt, in_=mb_d)

                xtp = []
                for j in range(4):
                    t = big.tile([128, 2 * S], F8, tag="big4", bufs=16,
                                 name=f"xtp{j}")
                    nc.sync.dma_start(out=t,
                                      in_=xtp_d[:, j * 2 * S:(j + 1) * 2 * S])
                    xtp.append(t.rearrange("p (a b) -> p a b", a=2))

                def loadw8(w_dram, name):
                    wts = []
                    for j in range(4):
                        wt = wst8(f"{name}w{j}")
                        nc.sync.dma_start(
                            out=wt,
                            in_=w_dram[:, j * 2048:(j + 1) * 2048].rearrange(
                                "p (a b) -> p a b", a=2))
                        wts.append(wt)
                    return wts

                def proj(w_dram, n_cols, out_tiles, name):
                    wts = loadw8(w_dram, name)
                    for mo in range(KI):
                        for cp in range(n_cols // 1024):
                            acc = scp(f"{name}ps{mo}_{cp}")
                            for j in range(4):
                                for c in range(2):
                                    cc = cp * 2 + c
                                    nc.tensor.matmul(
                                        acc[:, c * 512:(c + 1) * 512],
                                        wts[j][:, :, mo * 128:(mo + 1) * 128],
                                        xtp[j][:, :, cc * 512:(cc + 1) * 512],
                                        start=(j == 0), stop=(j == 3),
                                        perf_mode=DR,
                                    )
                            with nc.allow_low_precision(reason="bf16 proj"):
                                if (mo + cp) % 2 == 0:
                                    nc.vector.tensor_copy(
                                        out_tiles[mo][:, cp * 1024:(cp + 1) * 1024],
                                        acc[:, :])
                                else:
                                    nc.scalar.copy(
                                        out=out_tiles[mo][:, cp * 1024:(cp + 1) * 1024],
                                        in_=acc[:, :])

                qt = [b2k(f"qt{i}") for i in range(KI)]
                kt = [big4(f"kt{i}") for i in range(KI)]
                if ablate == "proj":
                    for t in qt:
                        nc.vector.memset(t, 0.01)
                    for t in kt:
                        nc.vector.memset(t, 0.01)
                else:
                    proj(wq_d, M, qt, "q")
                    proj(wk_d, S, kt, "k")

                wvts = []
                if ablate != "proj":
                    wvts = loadw8(wv_d, "vw")
                vt8 = []
                for kp in range(KT // 2):
                    v = big.tile([128, 2 * H * VST], F8, tag="vt",
                                 bufs=KT // 2, name=f"vt8_{kp}")
                    vt8.append(v.rearrange("p (a b) -> p a b", a=2))
                if ablate == "proj":
                    for v in vt8:
                        nc.vector.memset(v, 0.01)
                for k in (range(KT) if ablate != "proj" else []):
                    vacc = scp(f"vps{k}")
                    for j4 in range(4):
                        for c in range(2):
                            nc.tensor.matmul(
                                vacc[:, c * 512:(c + 1) * 512],
                                xtp[j4][:, :, k * 128:(k + 1) * 128],
                                wvts[j4][:, :, c * 512:(c + 1) * 512],
                                start=(j4 == 0), stop=(j4 == 3),
                                perf_mode=DR,
                            )
                    vslot = vt8[k // 2][:, k % 2, :]
                    for c in range(2):
                        acc = vacc[:, c * 512:(c + 1) * 512]
                        dst = vslot[:, c * 8 * VST:(c * 8 + 8) * VST].rearrange(
                            "p (h j) -> p h j", j=VST)[:, :, 0:DK]
                        src = acc.rearrange("p (h j) -> p h j", j=DK)
                        with nc.allow_low_precision(reason="v stored fp8"):
                            if k % 4 < 2:
                                nc.vector.tensor_copy(dst, src)
                            else:
                                nc.scalar.copy(out=dst, in_=src)
                    ones_view = vslot.rearrange(
                        "p (h j) -> p h j", j=VST)[:, :, DK:DK + 1]
                    nc.vector.memset(ones_view, 1.0)

                wots = loadw8(wo_d, "ow")

                ctxp8 = []
                for j4 in range(4):
                    t = mid.tile([128, 2048], F8, tag="b2k", bufs=17,
                                 name=f"ctxp8_{j4}")
                    ctxp8.append(t.rearrange("p (a b) -> p a b", a=2))
                if ablate == "attn":
                    for t in ctxp8:
                        nc.vector.memset(t, 0.01)
                pending_tail = [None]
                for hp in (range(H // 2) if ablate != "attn" else []):
                    heads = (2 * hp, 2 * hp + 1)
                    cps = [
                        [ps.tile([65, 512], PF32, tag="ctxp", bufs=2,
                                 name=f"cps{hp}_{c}") for c in range(QC)],
                        [accp(f"cpsb{hp}_{c}") for c in range(QC)],
                    ]
                    et8s = {}

                    def emit_ctx(kp):
                        for hi, h in enumerate(heads):
                            for c in range(QC):
                                nc.tensor.matmul(
                                    cps[hi][c][0:65, :],
                                    vt8[kp][:, :, h * VST:(h + 1) * VST],
                                    et8s[(kp, hi)][:, :, c * 512:(c + 1) * 512],
                                    start=(kp == 0), stop=(kp == KT // 2 - 1),
                                    perf_mode=DR,
                                )
                        del et8s[(kp, 0)], et8s[(kp, 1)]

                    for k in range(KT):
                        kp, ki2 = k // 2, k % 2
                        if ki2 == 0:
                            for hi in range(2):
                                t = mid.tile([128, 2048], F8, tag="scr",
                                             bufs=8, name=f"et8_{hp}_{kp}_{hi}")
                                et8s[(kp, hi)] = t.rearrange(
                                    "p (a b) -> p a b", a=2)
                        sps = [scp(f"sps{hp}_{k}_{hi}") for hi in range(2)]
                        for hi in range(2):
                            hb = hi * 64
                            for c in range(QC):
                                nc.tensor.matmul(
                                    sps[hi][:, c * 512:(c + 1) * 512],
                                    kt[hp][hb:hb + 64, k * 128:(k + 1) * 128],
                                    qt[hp][hb:hb + 64, c * 512:(c + 1) * 512],
                                    start=True, stop=True,
                                )
                        for hi in range(2):
                            nc.scalar.activation(
                                out=et8s[(kp, hi)][:, ki2, :], in_=sps[hi][:, :],
                                func=ACTF.Exp,
                                bias=(mbt[:, k:k + 1] if mask_has_zeros
                                      else 0.0),
                                scale=0.125,
                            )
                        if k == 2 and pending_tail[0] is not None:
                            pending_tail[0]()
                            pending_tail[0] = None
                        if ki2 == 1 and kp >= 1:
                            emit_ctx(kp - 1)
                    emit_ctx(KT // 2 - 1)
                    tail_data = []
                    for hi, h in enumerate(heads):
                        cc = mid.tile([65, 1024], F32, tag="tail", bufs=2,
                                      name=f"cc{hp}_{hi}")
                        for c in range(QC):
                            nc.vector.tensor_copy(
                                cc[:, c * 512:(c + 1) * 512],
                                cps[hi][c][0:65, :])
                        den = scr([1, 1024], BF, f"den{hp}_{hi}")
                        with nc.allow_low_precision(reason="bf16 softmax denom"):
                            nc.vector.reciprocal(out=den[:, :],
                                                 in_=cc[64:65, :])
                        tail_data.append((h, cc, den))

                    def _tail(tail_data=tail_data, hp=hp):
                        for h, cc, den in tail_data:
                            bcst = [accp(f"bcst{hp}_{h}_{c}")
                                    for c in range(QC)]
                            for c in range(QC):
                                nc.tensor.matmul(
                                    bcst[c][0:64, :],
                                    ones_row[:, :],
                                    den[:, c * 512:(c + 1) * 512],
                                    start=True, stop=True)
                            for c in range(QC):
                                with nc.allow_low_precision(
                                        reason="ctx stored fp8"):
                                    nc.vector.tensor_mul(
                                        ctxp8[h // 4][
                                            64 * (h % 2):64 * (h % 2) + 64,
                                            (h % 4) // 2,
                                            c * 512:(c + 1) * 512],
                                        cc[0:64, c * 512:(c + 1) * 512],
                                        bcst[c][0:64, :])

                    pending_tail[0] = _tail
                if pending_tail[0] is not None:
                    pending_tail[0]()
                    pending_tail[0] = None

                ln1_s1 = [accp(f"ln1s1_{c}") for c in range(QC)]
                ln1_s2 = [ps.tile([65, 512], PF32, tag="ctxp", bufs=2,
                                  name=f"ln1s2_{c}") for c in range(QC)]
                trunk = []
                for mo in range(KI):
                    xq = big.tile([128, 1024], F32, tag="xq", bufs=2, name=f"xq{mo}")
                    nc.sync.dma_start(out=xq, in_=xq32_d[mo * 128:(mo + 1) * 128, :])
                    tr = big4f(f"trunk{mo}")
                    trunk.append(tr)
                    aacc = scp(f"aops{mo}")
                    for j in range(4):
                        for c in range(QC):
                            nc.tensor.matmul(
                                aacc[:, c * 512:(c + 1) * 512],
                                wots[j][:, :, mo * 128:(mo + 1) * 128],
                                ctxp8[j][:, :, c * 512:(c + 1) * 512],
                                start=(j == 0), stop=(j == 3),
                                perf_mode=DR,
                            )
                    nc.vector.tensor_add(tr[:, :], aacc[:, :], xq[:, :])
                    sq = scr([128, 1024], BF, f"ln1sq{mo}")
                    nc.scalar.activation(out=sq[:, :], in_=tr[:, :],
                                         func=ACTF.Square)
                    for c in range(QC):
                        cs = slice(c * 512, (c + 1) * 512)
                        nc.tensor.matmul(
                            ln1_s1[c][0:1, :], ones_col32[:, :], tr[:, cs],
                            start=(mo == 0), stop=(mo == KI - 1))
                        nc.tensor.matmul(
                            ln1_s2[c][0:1, :], ones_col[:, :], sq[:, cs],
                            start=(mo == 0), stop=(mo == KI - 1))

                F32R = mybir.dt.float32r

                def layer_norm(g_idx, nm, s1=None, s2=None):
                    r_s1 = big.tile([1, 1024], F32R, tag="rows", bufs=3,
                                    name=f"{nm}rs1")
                    r_tmp = big.tile([1, 1024], F32R, tag="rows", bufs=3,
                                     name=f"{nm}rtmp")
                    r_istd = big.tile([1, 1024], F32R, tag="rows", bufs=3,
                                      name=f"{nm}ristd")
                    if s1 is None:
                        s1 = [accp(f"{nm}s1_{c}") for c in range(QC)]
                        for mo in range(KI):
                            for c in range(QC):
                                nc.tensor.matmul(
                                    s1[c][0:1, :], ones_col32[:, :],
                                    trunk[mo][:, c * 512:(c + 1) * 512],
                                    start=(mo == 0), stop=(mo == KI - 1))
                    for c in range(QC):
                        cs = slice(c * 512, (c + 1) * 512)
                        nc.vector.tensor_copy(r_s1[:, cs], s1[c][0:1, :])
                    if s2 is None:
                        s2 = [accp(f"{nm}s2_{c}") for c in range(QC)]
                        for mo in range(KI):
                            sq = scr([128, 1024], BF, f"{nm}sq{mo}")
                            nc.scalar.activation(out=sq[:, :],
                                                 in_=trunk[mo][:, :],
                                                 func=ACTF.Square)
                            for c in range(QC):
                                nc.tensor.matmul(
                                    s2[c][0:1, :], ones_col[:, :],
                                    sq[:, c * 512:(c + 1) * 512],
                                    start=(mo == 0), stop=(mo == KI - 1))
                    nc.scalar.activation(out=r_tmp[:, :], in_=r_s1[:, :],
                                         func=ACTF.Square)
                    for c in range(QC):
                        cs = slice(c * 512, (c + 1) * 512)
                        nc.vector.scalar_tensor_tensor(
                            out=r_tmp[:, cs], in0=r_tmp[:, cs],
                            scalar=-1.0 / D, in1=s2[c][0:1, :],
                            op0=AL.mult, op1=AL.add)
                    nc.scalar.activation(out=r_tmp[:, :], in_=r_tmp[:, :],
                                         func=ACTF.Sqrt, scale=1.0 / (D - 1))
                    nc.vector.reciprocal(out=r_istd[:, :], in_=r_tmp[:, :])
                    nc.vector.tensor_mul(r_s1[:, :], r_s1[:, :], r_istd[:, :])
                    nc.vector.tensor_scalar(
                        out=r_s1[:, :], in0=r_s1[:, :],
                        scalar1=cvec[:, 3 * g_idx + 1:3 * g_idx + 2],
                        scalar2=cvec[:, 3 * g_idx + 2:3 * g_idx + 3],
                        op0=AL.mult, op1=AL.add)
                    nc.vector.tensor_scalar_mul(
                        out=r_istd[:, :], in0=r_istd[:, :],
                        scalar1=cvec[:, 3 * g_idx:3 * g_idx + 1])
                    for c in range(QC):
                        cs = slice(c * 512, (c + 1) * 512)
                        bc = scp(f"{nm}bc{c}")
                        nc.tensor.matmul(
                            bc[:, 0:512], ones_row32[:, :],
                            r_istd[:, cs],
                            start=True, stop=True)
                        nc.tensor.matmul(
                            bc[:, 512:1024], ones_row32[:, :],
                            r_s1[:, cs],
                            start=True, stop=True)
                        for mo in range(KI):
                            nc.vector.tensor_mul(trunk[mo][:, cs],
                                                 trunk[mo][:, cs], bc[:, 0:512])
                            nc.vector.tensor_add(trunk[mo][:, cs],
                                                 trunk[mo][:, cs], bc[:, 512:1024])

                layer_norm(0, "ln1", s1=ln1_s1, s2=ln1_s2)

                KP = D // 256
                x2f8 = []
                for kp in range(KP):
                    t = mid.tile([128, 2048], F8, tag="b2k", bufs=17,
                                 name=f"x2f8_{kp}")
                    x2f8.append(t.rearrange("p (a b) -> p a b", a=2))
                for mo in range(KI):
                    with nc.allow_low_precision(reason="ffn input fp8"):
                        dst = x2f8[mo // 2][:, mo % 2, :]
                        if mo % 2 == 0:
                            nc.scalar.activation(
                                out=dst, in_=trunk[mo][:, :],
                                func=ACTF.Identity,
                                bias=cs1c[:, 32:33], scale=1.0)
                        else:
                            nc.vector.tensor_scalar(
                                out=dst, in0=trunk[mo][:, :],
                                scalar1=cs1c[:, 32:33], scalar2=None,
                                op0=AL.add)

                w1_v = w1_d.rearrange("p (a f) -> p a f", a=2 * KP)
                for g in (range(4) if ablate != "ffn" else []):
                    w1ts = []
                    for kp in range(KP):
                        wt = mid.tile([128, 2048], F8, tag="wst", bufs=20,
                                      name=f"w1t{g}_{kp}")
                        nc.sync.dma_start(
                            out=wt.rearrange("p (a b) -> p a b", a=2),
                            in_=w1_v[:, 2 * kp:2 * kp + 2,
                                     g * 1024:(g + 1) * 1024])
                        w1ts.append(wt.rearrange("p (a b) -> p a b", a=2))
                    ffb = []
                    for fl in range(8):
                        fb = scr([128, 1024], BF, f"ffb{g}_{fl}")
                        ffb.append(fb)
                        acc = scp(f"f1ps{g}_{fl}")
                        for kp in range(KP):
                            for c in range(QC):
                                nc.tensor.matmul(
                                    acc[:, c * 512:(c + 1) * 512],
                                    w1ts[kp][:, :, fl * 128:(fl + 1) * 128],
                                    x2f8[kp][:, :, c * 512:(c + 1) * 512],
                                    start=(kp == 0), stop=(kp == KP - 1),
                                    perf_mode=DR,
                                )
                        nc.scalar.activation(
                            out=fb[:, :], in_=acc[:, :], func=ACTF.Relu,
                            bias=cs1c[:, g * 8 + fl:g * 8 + fl + 1],
                            scale=1.0 / 16.0)
                    w2ts = []
                    for fl in range(8):
                        wt = wst(f"w2t{g}_{fl}")
                        nc.sync.dma_start(
                            out=wt,
                            in_=w2_d[(g * 8 + fl) * 128:(g * 8 + fl + 1) * 128, :])
                        w2ts.append(wt)
                    for mo in range(KI):
                        acc = [accp(f"f2ps{g}_{mo}_{c}") for c in range(QC)]
                        for fl in range(8):
                            for c in range(QC):
                                nc.tensor.matmul(
                                    acc[c][:, :],
                                    w2ts[fl][:, mo * 128:(mo + 1) * 128],
                                    ffb[fl][:, c * 512:(c + 1) * 512],
                                    start=(fl == 0), stop=(fl == 7),
                                )
                        for c in range(QC):
                            cs = slice(c * 512, (c + 1) * 512)
                            nc.vector.tensor_add(trunk[mo][:, cs],
                                                 trunk[mo][:, cs], acc[c][:, :])

                layer_norm(1, "ln2")

                for mo in range(KI):
                    nc.sync.dma_start(out=out_d[mo * 128:(mo + 1) * 128, :],
                                      in_=trunk[mo][:, :])

            for _rep in range(reps):
                with nc.allow_low_precision(
                        reason="f32r trunk: fp22 write rounding, verified "
                               "against fp32 reference"):
                    emit_body()

    nc.compile()
    return nc


def build2(mask_has_zeros: bool, reps: int = 1, ndev=N_CORES):
    import concourse.bass as bass
    import concourse.mybir as mybir
    import concourse.tile as tile
    from concourse import bacc
    import contextlib

    BF = mybir.dt.bfloat16
    F32 = mybir.dt.float32
    PF32 = mybir.dt.float32
    F32R = mybir.dt.float32r
    ACTF = mybir.ActivationFunctionType
    AL = mybir.AluOpType

    nc = bacc.Bacc("TRN2", target_bir_lowering=False, debug=False,
                   num_devices=ndev)

    F8 = mybir.dt.float8e4
    DR = mybir.MatmulPerfMode.DoubleRow
    xtp_d = nc.dram_tensor("xtp8", [128, 8 * S], F8, kind="ExternalInput").ap()
    xq32_d = nc.dram_tensor("xq32", [D, M], F32, kind="ExternalInput").ap()
    wq_d = nc.dram_tensor("wqp8", [128, 8192], F8, kind="ExternalInput").ap()
    wk_d = nc.dram_tensor("wkp8", [128, 8192], F8, kind="ExternalInput").ap()
    wv_d = nc.dram_tensor("wvp8", [128, 8192], F8, kind="ExternalInput").ap()
    wo_d = nc.dram_tensor("wop8", [128, 8192], F8, kind="ExternalInput").ap()
    w1_d = nc.dram_tensor("w1p8", [128, 2 * DFF * (D // 256)], F8,
                          kind="ExternalInput").ap()
    cs1_d = nc.dram_tensor("cs1c", [128, DFF // 128 + 1], F32,
                           kind="ExternalInput").ap()
    w2_d = nc.dram_tensor("w2b", [DFF, D], BF, kind="ExternalInput").ap()
    mb_d = nc.dram_tensor("maskb", [128, KT], F32, kind="ExternalInput").ap()
    cv_d = nc.dram_tensor("cvec", [1, 8], F32, kind="ExternalInput").ap()
    out_d = nc.dram_tensor("outT", [D, M], F32R, kind="ExternalOutput").ap()

    HM = M // 2
    KP = D // 256

    with tile.TileContext(nc) as tc:
        with contextlib.ExitStack() as ctx:
            big = ctx.enter_context(tc.tile_pool(name="big", bufs=1))
            mid = ctx.enter_context(tc.tile_pool(name="mid", bufs=1))
            ps = ctx.enter_context(tc.tile_pool(name="ps", bufs=1, space="PSUM"))

            def big4(name):
                return big.tile([128, 2048], BF, tag="big4", bufs=12, name=name)

            def trk(name):
                return big.tile([128, 512], F32R, tag="trk", bufs=12, name=name)

            def b2k(name):
                return mid.tile([128, 1024], BF, tag="b2k", bufs=8, name=name)

            def wst8(name):
                t = mid.tile([128, 2048], F8, tag="wst", bufs=12, name=name)
                return t.rearrange("p (a b) -> p a b", a=2)

            def wst(name):
                return mid.tile([128, 1024], BF, tag="wst", bufs=12, name=name)

            def scp(name):
                return ps.tile([128, 1024], PF32, tag="sc", bufs=2, name=name)

            def ctxp(rows, name):
                return ps.tile([rows, 512], PF32, tag="ctx", bufs=2, name=name)

            def trp(name):
                return ps.tile([128, 512], PF32, tag="tr", bufs=2, name=name)

            def emit_all():
                ones_row = mid.tile([1, 64], BF, tag="ones_r", bufs=1)
                nc.vector.memset(ones_row, 1.0)
                ones_col = mid.tile([128, 1], BF, tag="ones_c", bufs=1)
                nc.vector.memset(ones_col, 1.0)
                ones_f32 = mid.tile([128, 1], F32, tag="ones_f32", bufs=1)
                nc.vector.memset(ones_f32, 1.0)
                ones_rf32 = mid.tile([1, 128], F32, tag="ones_rf32", bufs=1)
                nc.vector.memset(ones_rf32, 1.0)
                ones_col32 = mid.tile([128, 1], F32R, tag="ones_c32", bufs=1)
                nc.vector.tensor_copy(ones_col32[:, :], ones_f32[:, :])
                ones_row32 = mid.tile([1, 128], F32R, tag="ones_r32", bufs=1)
                nc.vector.tensor_copy(ones_row32[:, :], ones_rf32[:, :])
                cvec = mid.tile([1, 8], F32, tag="cvec", bufs=1)
                nc.sync.dma_start(out=cvec, in_=cv_d)
                cs1c = mid.tile([128, DFF // 128 + 1], F32, tag="cs1c", bufs=1)
                nc.sync.dma_start(out=cs1c, in_=cs1_d)
                mbt = None
                if mask_has_zeros:
                    mbt = mid.tile([128, KT], F32, tag="mbt", bufs=1)
                    nc.sync.dma_start(out=mbt, in_=mb_d)

                def loadw8(w_dram, name, tag="wst", bufs=12):
                    wts = []
                    for j in range(4):
                        t = mid.tile([128, 2048], F8, tag=tag, bufs=bufs,
                                     name=f"{name}w{j}")
                        wt = t.rearrange("p (a b) -> p a b", a=2)
                        nc.sync.dma_start(
                            out=wt,
                            in_=w_dram[:, j * 2048:(j + 1) * 2048].rearrange(
                                "p (a b) -> p a b", a=2))
                        wts.append(wt)
                    return wts

                def emit_phase0(r):
                    xtp = []
                    for j in range(4):
                        t = big.tile([128, 2 * S], F8, tag="big4", bufs=12,
                                     name=f"xtp{j}_{r}")
                        nc.sync.dma_start(
                            out=t, in_=xtp_d[:, j * 2 * S:(j + 1) * 2 * S])
                        xtp.append(t.rearrange("p (a b) -> p a b", a=2))

                    def proj(w_dram, n_cols, out_tiles, name):
                        wts = loadw8(w_dram, name)
                        for mo in range(KI):
                            for cp in range(n_cols // 1024):
                                acc = scp(f"{name}ps{mo}_{cp}_{r}")
                                for j in range(4):
                                    for c in range(2):
                                        cc = cp * 2 + c
                                        nc.tensor.matmul(
                                            acc[:, c * 512:(c + 1) * 512],
                                            wts[j][:, :, mo * 128:(mo + 1) * 128],
                                            xtp[j][:, :, cc * 512:(cc + 1) * 512],
                                            start=(j == 0), stop=(j == 3),
                                            perf_mode=DR,
                                        )
                                with nc.allow_low_precision(reason="bf16 proj"):
                                    if (mo + cp) % 2 == 0:
                                        nc.vector.tensor_copy(
                                            out_tiles[mo][:, cp * 1024:(cp + 1) * 1024],
                                            acc[:, :])
                                    else:
                                        nc.scalar.copy(
                                            out=out_tiles[mo][:, cp * 1024:(cp + 1) * 1024],
                                            in_=acc[:, :])

                    qt = [b2k(f"qt{i}_{r}") for i in range(KI)]
                    kt = [big4(f"kt{i}_{r}") for i in range(KI)]
                    proj(wq_d, M, qt, "q")
                    proj(wk_d, S, kt, "k")

                    wvts = loadw8(wv_d, "vw")
                    vt8 = []
                    for kp in range(KT // 2):
                        v = big.tile([128, 2 * H * VST], F8, tag="vt",
                                     bufs=KT // 2, name=f"vt8_{kp}_{r}")
                        vt8.append(v.rearrange("p (a b) -> p a b", a=2))
                    for k in range(KT):
                        vacc = scp(f"vps{k}_{r}")
                        for j4 in range(4):
                            for c in range(2):
                                nc.tensor.matmul(
                                    vacc[:, c * 512:(c + 1) * 512],
                                    xtp[j4][:, :, k * 128:(k + 1) * 128],
                                    wvts[j4][:, :, c * 512:(c + 1) * 512],
                                    start=(j4 == 0), stop=(j4 == 3),
                                    perf_mode=DR,
                                )
                        vslot = vt8[k // 2][:, k % 2, :]
                        for c in range(2):
                            acc = vacc[:, c * 512:(c + 1) * 512]
                            dst = vslot[:, c * 8 * VST:(c * 8 + 8) * VST].rearrange(
                                "p (h j) -> p h j", j=VST)[:, :, 0:DK]
                            src = acc.rearrange("p (h j) -> p h j", j=DK)
                            with nc.allow_low_precision(reason="v stored fp8"):
                                if k % 4 < 2:
                                    nc.vector.tensor_copy(dst, src)
                                else:
                                    nc.scalar.copy(out=dst, in_=src)
                        ones_view = vslot.rearrange(
                            "p (h j) -> p h j", j=VST)[:, :, DK:DK + 1]
                        nc.vector.memset(ones_view, 1.0)

                    wots = loadw8(wo_d, "ow", tag="wot", bufs=8)
                    ctxp8 = []
                    for j4 in range(4):
                        t = mid.tile([128, 2048], F8, tag="cx8", bufs=8,
                                     name=f"ctxp8_{j4}_{r}")
                        ctxp8.append(t.rearrange("p (a b) -> p a b", a=2))
                    return dict(qt=qt, kt=kt, vt8=vt8, wots=wots,
                                ctxp8=ctxp8, cvec=cvec, cs1c=cs1c, mbt=mbt)

                def attn_half(st, h, r):
                    qt, kt, vt8 = st["qt"], st["kt"], st["vt8"]
                    ctxp8, mbt = st["ctxp8"], st["mbt"]
                    hs = slice(h * HM, (h + 1) * HM)
                    for hp in range(H // 2):
                        heads = (2 * hp, 2 * hp + 1)
                        cps = [ctxp(65 if hi == 0 else 128,
                                    f"cps{hp}_{hi}_{h}_{r}")
                               for hi in range(2)]
                        et8s = {}

                        def emit_ctx(kp):
                            for hi, hd in enumerate(heads):
                                nc.tensor.matmul(
                                    cps[hi][0:65, :],
                                    vt8[kp][:, :, hd * VST:(hd + 1) * VST],
                                    et8s[(kp, hi)],
                                    start=(kp == 0), stop=(kp == KT // 2 - 1),
                                    perf_mode=DR,
                                )
                            del et8s[(kp, 0)], et8s[(kp, 1)]

                        for kp in range(KT // 2):
                            sps = [scp(f"sps{hp}_{kp}_{hi}_{h}_{r}")
                                   for hi in range(2)]
                            for ki2 in range(2):
                                k = 2 * kp + ki2
                                for hi in range(2):
                                    hb = hi * 64
                                    nc.tensor.matmul(
                                        sps[hi][:, ki2 * 512:(ki2 + 1) * 512],
                                        kt[hp][hb:hb + 64, k * 128:(k + 1) * 128],
                                        qt[hp][hb:hb + 64, hs],
                                        start=True, stop=True,
                                    )
                            for hi in range(2):
                                t = mid.tile([128, 1024], F8, tag="et8",
                                             bufs=6, name=f"et8_{hp}_{kp}_{hi}_{h}_{r}")
                                if mask_has_zeros:
                                    for ki2 in range(2):
                                        nc.scalar.activation(
                                            out=t[:, ki2 * 512:(ki2 + 1) * 512],
                                            in_=sps[hi][:, ki2 * 512:(ki2 + 1) * 512],
                                            func=ACTF.Exp,
                                            bias=mbt[:, 2 * kp + ki2:2 * kp + ki2 + 1],
                                            scale=0.125)
                                else:
                                    nc.scalar.activation(
                                        out=t[:, :], in_=sps[hi][:, :],
                                        func=ACTF.Exp, bias=0.0, scale=0.125)
                                et8s[(kp, hi)] = t.rearrange(
                                    "p (a b) -> p a b", a=2)
                            if kp >= 1:
                                emit_ctx(kp - 1)
                        emit_ctx(KT // 2 - 1)
                        for hi, hd in enumerate(heads):
                            cc = mid.tile([65, 512], F32, tag="tail", bufs=3,
                                          name=f"cc{hp}_{hi}_{h}_{r}")
                            nc.vector.tensor_copy(cc[:, :], cps[hi][0:65, :])
                            den = mid.tile([1, 512], BF, tag="den", bufs=2,
                                           name=f"den{hp}_{hi}_{h}_{r}")
                            with nc.allow_low_precision(reason="bf16 denom"):
                                nc.vector.reciprocal(out=den[:, :],
                                                     in_=cc[64:65, :])
                            bc = ctxp(64, f"bcst{hp}_{hi}_{h}_{r}")
                            nc.tensor.matmul(bc[0:64, :], ones_row[:, :],
                                             den[:, :], start=True, stop=True)
                            with nc.allow_low_precision(reason="ctx fp8"):
                                nc.vector.tensor_mul(
                                    ctxp8[hd // 4][
                                        64 * (hd % 2):64 * (hd % 2) + 64,
                                        (hd % 4) // 2, hs],
                                    cc[0:64, :], bc[0:64, :])
                        yield

                def trunk_half(st, h, r):
                    wots, ctxp8 = st["wots"], st["ctxp8"]
                    cvec, cs1c = st["cvec"], st["cs1c"]
                    hs = slice(h * HM, (h + 1) * HM)
                    trunk = []
                    for mo in range(KI):
                        xq = mid.tile([128, 512], F32, tag="xq", bufs=2,
                                      name=f"xq{mo}_{h}_{r}")
                        nc.sync.dma_start(
                            out=xq, in_=xq32_d[mo * 128:(mo + 1) * 128, hs])
                        tr = trk(f"trunk{mo}_{h}_{r}")
                        trunk.append(tr)
                        aacc = trp(f"aops{mo}_{h}_{r}")
                        for j in range(4):
                            nc.tensor.matmul(
                                aacc[:, :],
                                wots[j][:, :, mo * 128:(mo + 1) * 128],
                                ctxp8[j][:, :, hs],
                                start=(j == 0), stop=(j == 3),
                                perf_mode=DR,
                            )
                        nc.vector.tensor_add(tr[:, :], aacc[:, :], xq[:, :])
                        if mo == 3:
                            yield
                    yield

                    def layer_norm(g_idx, nm):
                        s1 = trp(f"{nm}s1_{h}_{r}")
                        s2 = trp(f"{nm}s2_{h}_{r}")
                        for mo in range(KI):
                            nc.tensor.matmul(
                                s1[0:1, :], ones_col32[:, :], trunk[mo][:, :],
                                start=(mo == 0), stop=(mo == KI - 1))
                        for mo in range(KI):
                            sq = mid.tile([128, 512], BF, tag="sqr", bufs=2,
                                          name=f"{nm}sq{mo}_{h}_{r}")
                            nc.scalar.activation(out=sq[:, :],
                                                 in_=trunk[mo][:, :],
                                                 func=ACTF.Square)
                            nc.tensor.matmul(
                                s2[0:1, :], ones_col[:, :], sq[:, :],
                                start=(mo == 0), stop=(mo == KI - 1))
                        r_s1 = mid.tile([1, 512], F32R, tag="rows", bufs=4,
                                        name=f"{nm}rs1_{h}_{r}")
                        r_tmp = mid.tile([1, 512], F32R, tag="rows", bufs=4,
                                         name=f"{nm}rtmp_{h}_{r}")
                        r_istd = mid.tile([1, 512], F32R, tag="rows", bufs=4,
                                          name=f"{nm}ristd_{h}_{r}")
                        nc.vector.tensor_copy(r_s1[:, :], s1[0:1, :])
                        nc.scalar.activation(out=r_tmp[:, :], in_=r_s1[:, :],
                                             func=ACTF.Square)
                        nc.vector.scalar_tensor_tensor(
                            out=r_tmp[:, :], in0=r_tmp[:, :],
                            scalar=-1.0 / D, in1=s2[0:1, :],
                            op0=AL.mult, op1=AL.add)
                        nc.scalar.activation(out=r_tmp[:, :], in_=r_tmp[:, :],
                                             func=ACTF.Sqrt, scale=1.0 / (D - 1))
                        nc.vector.reciprocal(out=r_istd[:, :], in_=r_tmp[:, :])
                        nc.vector.tensor_mul(r_s1[:, :], r_s1[:, :], r_istd[:, :])
                        nc.vector.tensor_scalar(
                            out=r_s1[:, :], in0=r_s1[:, :],
                            scalar1=cvec[:, 3 * g_idx + 1:3 * g_idx + 2],
                            scalar2=cvec[:, 3 * g_idx + 2:3 * g_idx + 3],
                            op0=AL.mult, op1=AL.add)
                        nc.vector.tensor_scalar_mul(
                            out=r_istd[:, :], in0=r_istd[:, :],
                            scalar1=cvec[:, 3 * g_idx:3 * g_idx + 1])
                        bca = trp(f"{nm}bca_{h}_{r}")
                        bcb = trp(f"{nm}bcb_{h}_{r}")
                        nc.tensor.matmul(bca[:, :], ones_row32[:, :],
                                         r_istd[:, :], start=True, stop=True)
                        nc.tensor.matmul(bcb[:, :], ones_row32[:, :],
                                         r_s1[:, :], start=True, stop=True)
                        for mo in range(KI):
                            nc.vector.tensor_mul(trunk[mo][:, :],
                                                 trunk[mo][:, :], bca[:, :])
                            nc.vector.tensor_add(trunk[mo][:, :],
                                                 trunk[mo][:, :], bcb[:, :])

                    layer_norm(0, "ln1")
                    yield

                    x2f8 = []
                    for kp in range(KP):
                        t = mid.tile([128, 1024], F8, tag="x28", bufs=8,
                                     name=f"x2f8_{kp}_{h}_{r}")
                        x2f8.append(t.rearrange("p (a b) -> p a b", a=2))
                    for mo in range(KI):
                        with nc.allow_low_precision(reason="ffn input fp8"):
                            dst = x2f8[mo // 2][:, mo % 2, :]
                            if mo % 2 == 0:
                                nc.scalar.activation(
                                    out=dst, in_=trunk[mo][:, :],
                                    func=ACTF.Identity,
                                    bias=cs1c[:, 32:33], scale=1.0)
                            else:
                                nc.vector.tensor_scalar(
                                    out=dst, in0=trunk[mo][:, :],
                                    scalar1=cs1c[:, 32:33], scalar2=None,
                                    op0=AL.add)
                    yield

                    w1_v = w1_d.rearrange("p (a f) -> p a f", a=2 * KP)
                    for g in range(4):
                        w1ts = []
                        for kp in range(KP):
                            wt = mid.tile([128, 2048], F8, tag="wst", bufs=12,
                                          name=f"w1t{g}_{kp}_{h}_{r}")
                            nc.sync.dma_start(
                                out=wt.rearrange("p (a b) -> p a b", a=2),
                                in_=w1_v[:, 2 * kp:2 * kp + 2,
                                         g * 1024:(g + 1) * 1024])
                            w1ts.append(wt.rearrange("p (a b) -> p a b", a=2))
                        ffb = []
                        for fl in range(8):
                            fb = mid.tile([128, 512], BF, tag="ffb", bufs=9,
                                          name=f"ffb{g}_{fl}_{h}_{r}")
                            ffb.append(fb)
                            acc = trp(f"f1ps{g}_{fl}_{h}_{r}")
                            for kp in range(KP):
                                nc.tensor.matmul(
                                    acc[:, :],
                                    w1ts[kp][:, :, fl * 128:(fl + 1) * 128],
                                    x2f8[kp][:, :, :],
                                    start=(kp == 0), stop=(kp == KP - 1),
                                    perf_mode=DR,
                                )
                            nc.scalar.activation(
                                out=fb[:, :], in_=acc[:, :], func=ACTF.Relu,
                                bias=cs1c[:, g * 8 + fl:g * 8 + fl + 1],
                                scale=1.0 / 16.0)
                        w2ts = []
                        for fl in range(8):
                            wt = wst(f"w2t{g}_{fl}_{h}_{r}")
                            nc.sync.dma_start(
                                out=wt,
                                in_=w2_d[(g * 8 + fl) * 128:(g * 8 + fl + 1) * 128, :])
                            w2ts.append(wt)
                        for mo in range(KI):
                            acc = trp(f"f2ps{g}_{mo}_{h}_{r}")
                            for fl in range(8):
                                nc.tensor.matmul(
                                    acc[:, :],
                                    w2ts[fl][:, mo * 128:(mo + 1) * 128],
                                    ffb[fl][:, :],
                                    start=(fl == 0), stop=(fl == 7),
                                )
                            nc.vector.tensor_add(trunk[mo][:, :],
                                                 trunk[mo][:, :], acc[:, :])
                        yield

                    layer_norm(1, "ln2")
                    for mo in range(KI):
                        nc.sync.dma_start(
                            out=out_d[mo * 128:(mo + 1) * 128, hs],
                            in_=trunk[mo][:, :])
                    yield

                def interleave(gen_a, gen_t):
                    done_a = done_t = False
                    while not (done_a and done_t):
                        if not done_t:
                            done_t = next(gen_t, _SENTINEL) is _SENTINEL
                        if not done_a:
                            done_a = next(gen_a, _SENTINEL) is _SENTINEL

                pending = None
                for r in range(reps):
                    st = emit_phase0(r)
                    a0 = attn_half(st, 0, r)
                    if pending is not None:
                        interleave(a0, pending)
                    else:
                        for _ in a0:
                            pass
                    interleave(attn_half(st, 1, r), trunk_half(st, 0, r))
                    pending = trunk_half(st, 1, r)
                for _ in pending:
                    pass

            with nc.allow_low_precision(
                    reason="f32r trunk: fp22 write rounding, verified "
                           "against fp32 reference"):
                emit_all()

    nc.compile()
    return nc


_SENTINEL = object()


_NC_CACHE = {}


def _get_nc(mask_has_zeros: bool):
    if mask_has_zeros not in _NC_CACHE:
        _NC_CACHE[mask_has_zeros] = build2(mask_has_zeros)
    return _NC_CACHE[mask_has_zeros]


def _reference_numpy(x, mask, wq, bq, wk, bk, wv, bv, wo, bo, w1, b1, w2, b2,
                     g1, bt1, g2, bt2):
    import math
    out = np.zeros_like(x)

    def ln(v, g, bt):
        mean = v.mean(-1, keepdims=True)
        std = v.std(-1, keepdims=True, ddof=1)
        return g * ((v - mean) / std + EPS) + bt

    for b in range(B):
        xb = x[b]
        q = (xb @ wq + bq).reshape(S, H, DK).transpose(1, 0, 2)
        k = (xb @ wk + bk).reshape(S, H, DK).transpose(1, 0, 2)
        v = (xb @ wv + bv).reshape(S, H, DK).transpose(1, 0, 2)
        ctx = np.zeros((H, S, DK), np.float32)
        mrow = mask[b, 0, 0, :]
        for h in range(H):
            sc = (q[h] @ k[h].T) / math.sqrt(DK)
            sc = np.where(mrow[None, :] == 0, np.float32(-1e9), sc)
            e = np.exp(sc - sc.max(-1, keepdims=True))
            p = e / e.sum(-1, keepdims=True)
            ctx[h] = p @ v[h]
        cx = ctx.transpose(1, 0, 2).reshape(S, D)
        x1 = ln(xb + cx @ wo + bo, g1, bt1)
        ff = np.maximum(x1 @ w1 + b1, 0.0) @ w2 + b2
        out[b] = ln(x1 + ff, g2, bt2)
    return out


def kernel(**inputs) -> np.ndarray:
    from concourse.bass_utils import run_bass_kernel_spmd

    x = np.asarray(inputs["x"], np.float32)
    mask = np.asarray(inputs["mask"])
    wq, wk, wv, wo = (np.asarray(inputs[k], np.float32)
                      for k in ("wq", "wk", "wv", "wo"))
    w1 = np.asarray(inputs["w1"], np.float32)
    w2 = np.asarray(inputs["w2"], np.float32)
    g1 = float(np.asarray(inputs["g1"]))
    bt1 = float(np.asarray(inputs["bt1"]))
    g2 = float(np.asarray(inputs["g2"]))
    bt2 = float(np.asarray(inputs["bt2"]))
    biases = [np.asarray(inputs[k], np.float32)
              for k in ("bq", "bk", "bv", "bo", "b1", "b2")]

    if any(np.abs(b).max() > 0 for b in biases):
        return _reference_numpy(
            x, mask, wq, biases[0], wk, biases[1], wv, biases[2], wo,
            biases[3], w1, biases[4], w2, biases[5], g1, bt1, g2, bt2)

    mask_has_zeros = bool((mask == 0).any())
    nc = _get_nc(mask_has_zeros)
    in_maps = _prepare_in_maps(x, mask, wq, wk, wv, wo, w1, w2,
                               g1, bt1, g2, bt2)

    res = run_bass_kernel_spmd(nc, in_maps, core_ids=list(range(N_CORES)))
    globals()["LAST_RESULTS"] = res

    out = np.empty((B, S, D), np.float32)
    for core in range(N_CORES):
        b = core // 2
        qoff = (core % 2) * M
        out[b, qoff:qoff + M, :] = res.results[core]["outT"].T
    return out


F8NP = ml_dtypes.float8_e4m3


def _pair_rows(w):
    K_, F_ = w.shape
    t = w.reshape(K_ // 256, 2, 128, F_).transpose(2, 0, 1, 3)
    return np.ascontiguousarray(t.reshape(128, -1)).astype(F8NP)


def _prepare_in_maps(x, mask, wq, wk, wv, wo, w1, w2, g1, bt1, g2, bt2):
    wqp = _pair_rows(wq)
    wkp = _pair_rows(wk)
    wvp = _pair_rows(wv)
    wop = _pair_rows(wo)
    w1p = _pair_rows(w1 * 16.0)
    c1 = g1 * EPS + bt1
    cs1c = np.concatenate(
        [(c1 * w1.sum(0, dtype=np.float64)).astype(np.float32)
         .reshape(DFF // 128, 128).T,
         np.full((128, 1), -c1, np.float32)], axis=1)
    cs1c = np.ascontiguousarray(cs1c)
    w2b = w2.astype(NB)
    cvec = np.array([[g1, -g1 / D, g1 * EPS + bt1,
                      g2, -g2 / D, g2 * EPS + bt2, 0.0, 0.0]], np.float32)

    in_maps = []
    for core in range(N_CORES):
        b = core // 2
        qoff = (core % 2) * M
        xT = np.ascontiguousarray(x[b].T)
        mrow = np.where(mask[b, 0, 0, :] == 0, np.float32(-1e9),
                        np.float32(0.0)).astype(np.float32)
        if qoff:
            xT_k = np.concatenate([xT[:, M:], xT[:, :M]], axis=1)
            mrow = np.concatenate([mrow[M:], mrow[:M]])
        else:
            xT_k = xT
        in_maps.append({
            "xtp8": _pair_rows(xT_k),
            "xq32": np.ascontiguousarray(xT[:, qoff:qoff + M]),
            "wqp8": wqp, "wkp8": wkp, "wvp8": wvp, "wop8": wop,
            "w1p8": w1p, "cs1c": cs1c, "w2b": w2b,
            "maskb": np.ascontiguousarray(mrow.reshape(KT, 128).T),
            "cvec": cvec,
        })
    return in_maps


if __name__ == "__main__":
    d = np.load("/root/problem/ref_cache.npz")
    inputs = {k: d[k] for k in d.files if k != "exp"}
    got = kernel(**inputs)
    exp = d["exp"]
    err = np.abs(got - exp)
    print("max abs err:", err.max())
    print("rel max:", err.max() / np.abs(exp).max())
    print("rel l2:", np.linalg.norm(err) / np.linalg.norm(exp))

